# revision 6
# baseline (speedup 1.0000x reference)
"""BEVFormer block for Trainium2 — 8-core SPMD Bass kernel.

Strategy: all matmuls (the dominant FLOPs: QKV/offset/attention projections,
camera value projection, output projections, FFN — ~11.6 GFLOP) run on the 8
NeuronCores through one compiled Bass kernel with rows sharded across cores
(data-parallel over BEV queries / camera tokens). The kernel computes
OUT[2688,512] = X[2688,512] @ W[512,512] per core in bf16 with f32 PSUM
accumulation. Host does layernorm/softmax/bilinear-sampling glue in numpy.
"""
import numpy as np
import ml_dtypes

import concourse.bass as bass
import concourse.bacc as bacc
import concourse.mybir as mybir
import concourse.tile as tile
from concourse import bass_utils

EMBED = 256; NH = 8; HD = 32
SA_NP = 4; CA_NP = 8; NCAM = 6
BEV_H = 100; BEV_W = 100; ZSIZE = 8; DPIL = 4
PC = [-50.0, -50.0, -10.0, 50.0, 50.0, 10.0]
IMG_H = 224; IMG_W = 480
FH = 28; FW = 60
NQ = BEV_H * BEV_W
NCORES = 8
ROW_SIZES = (1280, 2688)      # per-core padded row options (10 / 21 tiles)
KDIM = 512
MDIM = 512

_CACHED_NC = {}
_CACHED_FN = {}


def _build_nc(rows):
    """One Bass kernel: OUT = XT.T @ W, XT [512, rows] bf16, W [512, 512] bf16."""
    dt = mybir.dt
    nc = bacc.Bacc("TRN2", target_bir_lowering=False, debug=False)
    xt_d = nc.dram_tensor("xt", [KDIM, rows], dt.bfloat16, kind="ExternalInput")
    w_d = nc.dram_tensor("w", [KDIM, MDIM], dt.bfloat16, kind="ExternalInput")
    out_d = nc.dram_tensor("out", [rows, MDIM], dt.bfloat16, kind="ExternalOutput")
    ntiles = rows // 128
    with tile.TileContext(nc) as tc:
        with tc.tile_pool(name="xt", bufs=1) as xpool, \
             tc.tile_pool(name="w", bufs=1) as wpool, \
             tc.tile_pool(name="res", bufs=4) as rpool, \
             tc.tile_pool(name="psum", bufs=4, space="PSUM") as ppool:
            xts = []
            ws = []
            for kt in range(KDIM // 128):
                xtt = xpool.tile([128, rows], dt.bfloat16, tag=f"x{kt}")
                nc.sync.dma_start(xtt[:], xt_d[kt * 128:(kt + 1) * 128, :])
                xts.append(xtt)
                wt = wpool.tile([128, MDIM], dt.bfloat16, tag=f"w{kt}")
                nc.sync.dma_start(wt[:], w_d[kt * 128:(kt + 1) * 128, :])
                ws.append(wt)
            for mt in range(ntiles):
                psum = ppool.tile([128, MDIM], dt.float32, tag="p")
                for kt in range(KDIM // 128):
                    nc.tensor.matmul(
                        psum[:], xts[kt][:, mt * 128:(mt + 1) * 128], ws[kt][:],
                        start=(kt == 0), stop=(kt == KDIM // 128 - 1))
                res = rpool.tile([128, MDIM], dt.bfloat16, tag="r")
                nc.vector.tensor_copy(res[:], psum[:])
                nc.sync.dma_start(out_d[mt * 128:(mt + 1) * 128, :], res[:])
    nc.compile()
    return nc


def _build_sharded_fn(nc, rows):
    """Cache the jitted 8-core shard_map executable (mirrors
    bass2jax.run_bass_via_pjrt) so repeated calls skip re-lowering."""
    import jax
    from jax.experimental.shard_map import shard_map
    from jax.sharding import Mesh, PartitionSpec
    from concourse import bass2jax
    bass2jax.install_neuronx_cc_hook()

    out_avals = [jax.core.ShapedArray((rows, MDIM), ml_dtypes.bfloat16)]
    in_names = ("xt", "w", "out")
    out_names = ("out",)

    import jax.numpy as jnp

    def _body(xt, w):
        zeros = jnp.zeros((rows, MDIM), ml_dtypes.bfloat16)
        outs = bass2jax._bass_exec_p.bind(
            xt, w, zeros,
            out_avals=tuple(out_avals),
            in_names=in_names,
            out_names=out_names,
            lowering_input_output_aliases=(),
            sim_require_finite=True,
            sim_require_nnan=True,
            nc=nc,
        )
        return tuple(outs)

    devices = jax.devices()[:NCORES]
    mesh = Mesh(np.asarray(devices), ("core",))
    in_specs = (PartitionSpec("core"),) * 2
    out_specs = (PartitionSpec("core"),)
    return jax.jit(
        shard_map(_body, mesh=mesh, in_specs=in_specs, out_specs=out_specs,
                  check_rep=False),
        keep_unused=True)


def _device_mm(X, W):
    """X [N, K<=512] @ W [K<=512, M<=512] on 8 NeuronCores, rows sharded."""
    N, K = X.shape
    M = W.shape[1]
    rows = next(r for r in ROW_SIZES if N <= r * NCORES)
    if rows not in _CACHED_NC:
        _CACHED_NC[rows] = _build_nc(rows)
    nc = _CACHED_NC[rows]
    NTOT = rows * NCORES
    assert K <= KDIM and M <= MDIM
    Xp = np.zeros((NTOT, KDIM), np.float32)
    Xp[:N, :K] = X
    Wp = np.zeros((KDIM, MDIM), np.float32)
    Wp[:K, :M] = W
    Wb = Wp.astype(ml_dtypes.bfloat16)
    xts = np.concatenate(
        [np.ascontiguousarray(Xp[c * rows:(c + 1) * rows].T)
         for c in range(NCORES)], axis=0).astype(ml_dtypes.bfloat16)
    ws = np.concatenate([Wb] * NCORES, axis=0)
    try:
        if rows not in _CACHED_FN:
            _CACHED_FN[rows] = _build_sharded_fn(nc, rows)
        (out_all,) = _CACHED_FN[rows](xts, ws)
        out = np.asarray(out_all).astype(np.float32).reshape(NTOT, MDIM)
    except Exception:
        in_maps = []
        for c in range(NCORES):
            in_maps.append(
                {"xt": xts[c * KDIM:(c + 1) * KDIM], "w": Wb})
        res = bass_utils.run_bass_kernel_spmd(
            nc, in_maps, core_ids=list(range(NCORES)), trace=False)
        out = np.concatenate(
            [r["out"].astype(np.float32) for r in res.results], axis=0)
    return out[:N, :M]


# ---------------- host-side glue (layernorm / softmax / sampling) ----------

def _layer_norm(x, g, b, eps=1e-5):
    mu = x.mean(-1, keepdims=True)
    var = ((x - mu) ** 2).mean(-1, keepdims=True)
    return (x - mu) / np.sqrt(var + eps) * g + b


def _softmax(x, axis):
    m = x.max(axis=axis, keepdims=True)
    e = np.exp(x - m)
    return e / e.sum(axis=axis, keepdims=True)


def _ms_deform_attn(value, H, W, loc, attn):
    Bv, L, nh, hd = value.shape
    Nq = loc.shape[1]; P = loc.shape[3] * loc.shape[4]
    v = np.transpose(value, (0, 2, 1, 3))
    loc = loc.reshape(Bv, Nq, nh, P, 2)
    attn = attn.reshape(Bv, Nq, nh, P)
    x = loc[..., 0] * W - 0.5
    y = loc[..., 1] * H - 0.5
    x0 = np.floor(x); y0 = np.floor(y)
    fx = x - x0; fy = y - y0
    x0i = x0.astype(np.int32); y0i = y0.astype(np.int32)
    out = np.zeros((Bv, nh, Nq, hd), value.dtype)
    for dx, dy, w in ((0, 0, (1 - fx) * (1 - fy)), (1, 0, fx * (1 - fy)),
                      (0, 1, (1 - fx) * fy), (1, 1, fx * fy)):
        xi = x0i + dx; yi = y0i + dy
        valid = ((xi >= 0) & (xi < W) & (yi >= 0) & (yi < H)).astype(value.dtype)
        idx = np.clip(yi, 0, H - 1) * W + np.clip(xi, 0, W - 1)
        idxt = np.transpose(idx, (0, 2, 1, 3)).reshape(Bv, nh, Nq * P)
        g = np.take_along_axis(v, idxt[..., None], axis=2).reshape(Bv, nh, Nq, P, hd)
        out = out + np.einsum('bhqpd,bqhp->bhqd', g, w * valid * attn)
    return np.transpose(out, (0, 2, 1, 3)).reshape(Bv, Nq, nh * hd)


def _ref_points_3d():
    zs = (np.linspace(0.5, ZSIZE - 0.5, DPIL, dtype=np.float32) / ZSIZE)[:, None, None]
    ys = ((np.arange(BEV_H, dtype=np.float32) + 0.5) / BEV_H)[None, :, None]
    xs = ((np.arange(BEV_W, dtype=np.float32) + 0.5) / BEV_W)[None, None, :]
    shp = (DPIL, BEV_H, BEV_W)
    ref = np.stack([np.broadcast_to(xs, shp), np.broadcast_to(ys, shp),
                    np.broadcast_to(zs, shp)], -1)
    return ref.reshape(1, DPIL, NQ, 3)


def _point_sampling(ref3d, lidar2img):
    pc = np.asarray(PC, np.float32)
    xyz = np.stack([ref3d[..., 0] * (pc[3] - pc[0]) + pc[0],
                    ref3d[..., 1] * (pc[4] - pc[1]) + pc[1],
                    ref3d[..., 2] * (pc[5] - pc[2]) + pc[2]], -1)
    pts = np.concatenate([xyz, np.ones_like(xyz[..., :1])], -1)
    proj = np.einsum('bnij,bdqj->bndqi', lidar2img, pts)
    eps = 1e-5
    z = proj[..., 2:3]
    mask = proj[..., 2] > eps
    xy = proj[..., :2] / np.maximum(z, eps)
    xy = xy / np.asarray([IMG_W, IMG_H], np.float32)
    mask = mask & (xy[..., 0] > 0.0) & (xy[..., 0] < 1.0) & \
        (xy[..., 1] > 0.0) & (xy[..., 1] < 1.0)
    ref_cam = np.transpose(xy, (1, 0, 3, 2, 4))
    bev_mask = np.transpose(mask, (1, 0, 3, 2))
    return ref_cam, bev_mask


def kernel(x, bev_pos, camera_feats, lidar2img, params):
    p = {k: np.asarray(v, np.float32) for k, v in params.items()}
    x = np.asarray(x, np.float32)
    bev_pos = np.asarray(bev_pos, np.float32)
    camera_feats = np.asarray(camera_feats, np.float32)
    lidar2img = np.asarray(lidar2img, np.float32)
    x2 = x[0]; pos2 = bev_pos[0]

    # ---- SA projections (device): rows [h; q] @ [Wv | Woff | Wattn] ----
    h = _layer_norm(x2, p['ln1_g'], p['ln1_b'])
    q = h + pos2
    W1 = np.concatenate([p['sa_Wv'], p['sa_Woff'], p['sa_Wattn']], axis=1)  # 256x352
    O1 = _device_mm(np.concatenate([h, q], axis=0), W1)
    v = (O1[:NQ, 0:256] + p['sa_bv']).reshape(1, NQ, NH, HD)
    off = (O1[NQ:2 * NQ, 256:320] + p['sa_boff']).reshape(1, NQ, NH, 1, SA_NP, 2)
    aw = _softmax((O1[NQ:2 * NQ, 320:352] + p['sa_battn']).reshape(1, NQ, NH, SA_NP), -1)
    aw = aw.reshape(1, NQ, NH, 1, SA_NP)

    ys = (np.arange(BEV_H, dtype=np.float32) + 0.5) / BEV_H
    xs = (np.arange(BEV_W, dtype=np.float32) + 0.5) / BEV_W
    gy, gx = np.meshgrid(ys, xs, indexing='ij')
    ref2d = np.stack([gx.ravel(), gy.ravel()], -1)
    loc = ref2d[None, :, None, None, None, :] + off / np.asarray([BEV_W, BEV_H], np.float32)
    sa = _ms_deform_attn(v, BEV_H, BEV_W, loc, aw)[0]

    # ---- SA out projection (device) + residual ----
    h1 = (_device_mm(sa, p['sa_Wout']) + p['sa_bout']) + x2

    # ---- cross attention ----
    feats = np.transpose(camera_feats, (1, 0, 3, 4, 2)).reshape(NCAM, FH * FW, EMBED)
    feats = feats + p['camera_emb'][:, None, :] + p['level_emb'][0]
    ref_cam, bev_mask = _point_sampling(_ref_points_3d(), lidar2img)
    hq = _layer_norm(h1, p['ln2_g'], p['ln2_b'])

    W3 = np.concatenate([p['ca_Woff'], p['ca_Wattn'], p['ca_Wv']], axis=1)  # 256x448
    rows3 = np.concatenate([hq, feats.reshape(NCAM * FH * FW, EMBED)], axis=0)
    O3 = _device_mm(rows3, W3)
    off_c = (O3[:NQ, 0:128] + p['ca_boff'])
    aw_c = _softmax((O3[:NQ, 128:192] + p['ca_battn']).reshape(NQ, NH, CA_NP), -1)
    vv = (O3[NQ:NQ + NCAM * FH * FW, 192:448] + p['ca_bv'])
    vv = vv.reshape(NCAM, FH * FW, NH, HD)

    NB = NCAM; L = FH * FW
    off_b = np.broadcast_to(off_c[None], (NB, NQ, 128)).reshape(NB, NQ, NH, 1, CA_NP, 2)
    aw_b = np.broadcast_to(aw_c[None], (NB, NQ, NH, CA_NP)).reshape(NB, NQ, NH, 1, CA_NP)
    off_n = (off_b / np.asarray([FW, FH], np.float32)).reshape(
        NB, NQ, NH, 1, CA_NP // DPIL, DPIL, 2)
    ref = ref_cam.reshape(NB, NQ, DPIL, 2)
    loc_c = (ref[:, :, None, None, None, :, :] + off_n).reshape(NB, NQ, NH, 1, CA_NP, 2)
    ca = _ms_deform_attn(vv, FH, FW, loc_c, aw_b).reshape(NCAM, NQ, EMBED)
    hit = np.any(bev_mask, -1).astype(np.float32)[:, 0]      # (NCAM, NQ)
    slots = np.sum(ca * hit[..., None], 0)
    count = np.maximum(hit.sum(0), 1.0)
    slots = slots / count[..., None]

    # ---- CA out projection (device) + residuals ----
    h2 = (_device_mm(slots, p['ca_Wout']) + p['ca_bout']) + hq + h1

    # ---- FFN (device matmuls) ----
    hf = _layer_norm(h2, p['ln3_g'], p['ln3_b'])
    f1 = np.maximum(_device_mm(hf, p['ffn_W1']) + p['ffn_b1'], 0.0)
    f2 = _device_mm(f1, p['ffn_W2']) + p['ffn_b2']
    out = (hf + f2) + h2
    return out[None].astype(np.float32)
